# revision 2
# baseline (speedup 1.0000x reference)
"""Trainium2 Bass kernel for the DAG-conditional gated GNN encoder.

8-core SPMD. Edges are partitioned by destination node (dest-sharding):
core c owns nodes [c*VS,(c+1)*VS) and every edge targeting them, so the
segment-sum aggregation is core-local (one-hot matmuls accumulating in
PSUM on top of the Uh contribution). Feature-major compute everywhere a
matmul is involved; gathered rows (edge-major) are pivoted with xbar
DMA transposes. BatchNorm batch statistics are accumulated on the fly
(ACT accum_out / STS accum_out / ones-augmented Gram matmuls) and
combined across cores with one tiny AllReduce collective.

kernel(**inputs) takes the full unsharded inputs and returns the full
(x_out, e_out) tuple, matching reference.reference().
"""

from contextlib import ExitStack

import numpy as np
import ml_dtypes

import concourse.bass as bass
import concourse.tile as tile
from concourse import bacc, mybir
from concourse.masks import make_identity
from concourse.tile import add_dep_helper
from concourse.bass_utils import run_bass_kernel_spmd

BF16 = np.dtype(ml_dtypes.bfloat16)
BF = mybir.dt.bfloat16
F32 = mybir.dt.float32
I32 = mybir.dt.int32
I16 = mybir.dt.int16
AL = mybir.AluOpType
AF = mybir.ActivationFunctionType
F = np.float32

H = 128
CHUNK = 128
GROUP = 4
N_CORES = 8
EPS = 1e-5


def _bf16(x):
    return np.asarray(x, dtype=np.float32).astype(BF16)


# ----------------------------------------------------------------------
# host-side plan
# ----------------------------------------------------------------------
class _Core:
    pass


class _Plan:
    pass


def _build_plan(edge_index, V):
    row = np.asarray(edge_index[0], dtype=np.int64)
    col = np.asarray(edge_index[1], dtype=np.int64)
    E = row.shape[0]
    VS = V // N_CORES
    NT = (VS + 127) // 128
    VT_PAD = NT * 128

    order = np.argsort(row, kind="stable")
    srow = row[order]
    shard_bounds = np.searchsorted(srow, np.arange(N_CORES + 1) * VS)
    counts = np.zeros((N_CORES, NT), dtype=np.int64)
    for c in range(N_CORES):
        lo, hi = shard_bounds[c], shard_bounds[c + 1]
        lt = (srow[lo:hi] - c * VS) // 128
        counts[c] = np.bincount(lt, minlength=NT)
    tile_chunks = np.maximum((counts.max(axis=0) + CHUNK - 1) // CHUNK, 1)
    n_chunks = int(tile_chunks.sum())
    E_pad = n_chunks * CHUNK

    p = _Plan()
    p.V, p.E, p.VS, p.NT, p.VT_PAD = V, E, VS, NT, VT_PAD
    p.tile_chunks, p.n_chunks, p.E_pad = tile_chunks, n_chunks, E_pad
    p.cores = []
    tcs = np.concatenate([[0], np.cumsum(tile_chunks)])
    for c in range(N_CORES):
        cp = _Core()
        lo, hi = shard_bounds[c], shard_bounds[c + 1]
        eids = order[lo:hi]
        lr = (row[eids] - c * VS).astype(np.int64)
        lt = lr // 128
        col_idx = np.full(E_pad, V, dtype=np.int32)
        lr_idx = np.full(E_pad, VT_PAD, dtype=np.int32)
        r_rel = np.full(E_pad, -1.0, dtype=np.float32)
        orig = np.full(E_pad, -1, dtype=np.int64)
        start = np.searchsorted(lt, np.arange(NT + 1))
        for g in range(NT):
            a, b = start[g], start[g + 1]
            base = tcs[g] * CHUNK
            col_idx[base : base + b - a] = col[eids[a:b]]
            lr_idx[base : base + b - a] = lr[a:b]
            r_rel[base : base + b - a] = (lr[a:b] - 128 * g).astype(np.float32)
            orig[base : base + b - a] = eids[a:b]
        cp.col_idx = np.ascontiguousarray(col_idx.reshape(n_chunks, CHUNK).T)
        cp.lr_idx = np.ascontiguousarray(lr_idx.reshape(n_chunks, CHUNK).T)
        cp.r_rel = np.ascontiguousarray(r_rel.reshape(n_chunks, CHUNK).T)
        cp.orig = orig
        p.cores.append(cp)
    return p


# ----------------------------------------------------------------------
# device program
# ----------------------------------------------------------------------
def _build_bass(meta):
    V, E = meta["V"], meta["E"]
    NT, VT_PAD = meta["NT"], meta["VT_PAD"]
    n_chunks, E_pad = meta["n_chunks"], meta["E_pad"]
    tile_chunks = meta["tile_chunks"]

    nc = bacc.Bacc("TRN2", target_bir_lowering=False, debug=False,
                   num_devices=N_CORES)

    def din(name, shape, dt):
        return nc.dram_tensor(name, shape, dt, kind="ExternalInput").ap()

    def dout(name, shape, dt):
        return nc.dram_tensor(name, shape, dt, kind="ExternalOutput").ap()

    eT_d = din("eT", [H, E_pad], BF)
    h_ext_d = din("h_ext", [V + 8, H], BF)
    hT_loc_d = din("hT_loc", [H, VT_PAD], BF)
    h_loc_d = din("h_loc", [VT_PAD, H], F32)
    col_d = din("col_idx", [CHUNK, n_chunks], I32)
    lr_d = din("lr_idx", [CHUNK, n_chunks], I32)
    rrel_d = din("r_rel", [CHUNK, n_chunks], F32)
    w_d = {k: din(k, [H, H], BF) for k in ["WU", "WV", "WA", "WB", "WC", "Wo"]}
    vecs_d = din("vecs", [H, 8], F32)

    xout_d = dout("x_out", [VT_PAD, H], F32)
    eoutT_d = dout("eoutT", [H, E_pad], F32)

    bh_d = nc.dram_tensor("bh_tab", [VT_PAD + CHUNK, H], BF).ap()
    epre_d = nc.dram_tensor("epre", [H, E_pad], BF).ap()
    hn_d = nc.dram_tensor("hn", [VT_PAD, H], BF).ap()
    stats_in_d = nc.dram_tensor("stats_in", [H, 4], F32).ap()
    stats_out_d = nc.dram_tensor("stats_out", [H, 4], F32).ap()

    groups = []
    tcs = np.concatenate([[0], np.cumsum(tile_chunks)]).astype(int)
    for g in range(NT):
        c0, c1 = tcs[g], tcs[g + 1]
        ch = c0
        while ch < c1:
            k = min(GROUP, c1 - ch)
            groups.append((g, ch, k, ch == c0, ch + k == c1))
            ch += k
    n_groups = len(groups)

    with tile.TileContext(nc) as tc, ExitStack() as ctx:
        cpool = ctx.enter_context(tc.tile_pool(name="const", bufs=1))
        wpool = ctx.enter_context(tc.tile_pool(name="work", bufs=3))
        gpool = ctx.enter_context(tc.tile_pool(name="gath", bufs=3))
        opool = ctx.enter_context(tc.tile_pool(name="outp", bufs=3))
        pp = ctx.enter_context(tc.tile_pool(name="ps_p", bufs=2, space="PSUM"))
        pq = ctx.enter_context(tc.tile_pool(name="ps_q", bufs=2, space="PSUM"))
        ph = ctx.enter_context(tc.tile_pool(name="ps_h", bufs=2, space="PSUM"))
        ps = ctx.enter_context(tc.tile_pool(name="ps_s", bufs=1, space="PSUM"))

        w_t = {}
        for k, d in w_d.items():
            w_t[k] = cpool.tile([H, H], BF, tag=f"w_{k}", name=f"w_{k}")
            nc.sync.dma_start(w_t[k][:], d[:, :])
        vecs_t = cpool.tile([H, 8], F32, tag="vecs")
        nc.sync.dma_start(vecs_t[:], vecs_d[:, :])
        ceb, bV, c2 = vecs_t[:, 0:1], vecs_t[:, 1:2], vecs_t[:, 2:3]
        g_e, b_e = vecs_t[:, 3:4], vecs_t[:, 4:5]
        g_h, b_h = vecs_t[:, 5:6], vecs_t[:, 6:7]
        eps_v = vecs_t[:, 7:8]

        col_t = cpool.tile([CHUNK, n_chunks], I32, tag="colidx")
        nc.sync.dma_start(col_t[:], col_d[:, :])
        lr_t = cpool.tile([CHUNK, n_chunks], I32, tag="lridx")
        nc.sync.dma_start(lr_t[:], lr_d[:, :])
        rrel_t = cpool.tile([CHUNK, n_chunks], F32, tag="rrel")
        nc.sync.dma_start(rrel_t[:], rrel_d[:, :])
        hT_all = cpool.tile([H, VT_PAD], BF, tag="hT_all")
        nc.sync.dma_start(hT_all[:], hT_loc_d[:, :])

        iota_i = cpool.tile([H, H], I16, tag="iota_i")
        nc.gpsimd.iota(iota_i[:], pattern=[[1, H]], base=0, channel_multiplier=0)
        iota_bf = cpool.tile([H, H], BF, tag="iota_bf")
        nc.vector.tensor_copy(iota_bf[:], iota_i[:])
        ident_bf = cpool.tile([H, H], BF, tag="ident_bf")
        make_identity(nc, ident_bf[:])
        ident_f = cpool.tile([H, H], F32, tag="ident_f")
        make_identity(nc, ident_f[:])

        esum_strip = cpool.tile([H, n_groups], F32, tag="esum_strip")
        esq_strip = cpool.tile([H, n_groups], F32, tag="esq_strip")

        # phase N: Bh table
        zero_bf = cpool.tile([CHUNK, H], BF, tag="zero_bf")
        nc.vector.memset(zero_bf[:], 0.0)
        bh_writes = [nc.sync.dma_start(bh_d[VT_PAD : VT_PAD + CHUNK, :],
                                       zero_bf[:]).ins]
        for g in range(NT):
            bh_ps = pq.tile([H, 512], F32, tag="q")
            nc.tensor.matmul(bh_ps[:, :H], lhsT=hT_all[:, g * H : (g + 1) * H],
                             rhs=w_t["WB"][:], start=True, stop=True)
            bh_sb = wpool.tile([CHUNK, H], BF, tag="bh_sb")
            nc.scalar.activation(bh_sb[:], bh_ps[:, :H], AF.Copy)
            bh_writes.append(
                nc.scalar.dma_start(bh_d[g * H : (g + 1) * H, :], bh_sb[:]).ins)
        bh_join = nc.sync.nop(nofuse=True, hint="bh_join").ins
        for w in bh_writes:
            add_dep_helper(bh_join, w, reason="bh table before gathers")

        # phase E
        epre_writes = []
        hn_writes = [None] * NT
        hn_ps = None
        hstat_ps = ps.tile([H, 256], F32, tag="hstat")
        for gi, (g, ch0, k, first, last) in enumerate(groups):
            N = k * CHUNK
            j0 = ch0 * CHUNK
            if first:
                hn_ps = ph.tile([H, 512], F32, tag="hn")
                nc.tensor.matmul(hn_ps[:, :H],
                                 lhsT=hT_all[:, g * H : (g + 1) * H],
                                 rhs=w_t["WU"][:], start=True, stop=False)
            eT_t = wpool.tile([H, 512], BF, tag="eT")
            nc.sync.dma_start(eT_t[:, :N], eT_d[:, j0 : j0 + N])

            hG = gpool.tile([CHUNK, 512], BF, tag="hG")
            bhG = gpool.tile([CHUNK, 512], BF, tag="bhG")
            hGT = gpool.tile([H, 512], BF, tag="hGT")
            bhGT = gpool.tile([H, 512], BF, tag="bhGT")
            for c in range(k):
                sl = slice(c * CHUNK, (c + 1) * CHUNK)
                nc.gpsimd.indirect_dma_start(
                    out=hG[:, sl], out_offset=None, in_=h_ext_d[:, :],
                    in_offset=bass.IndirectOffsetOnAxis(
                        ap=col_t[:, ch0 + c : ch0 + c + 1], axis=0))
                i2 = nc.gpsimd.indirect_dma_start(
                    out=bhG[:, sl], out_offset=None, in_=bh_d[:, :],
                    in_offset=bass.IndirectOffsetOnAxis(
                        ap=lr_t[:, ch0 + c : ch0 + c + 1], axis=0))
                add_dep_helper(i2.ins, bh_join, reason="gather after bh")
                nc.sync.dma_start_transpose(hGT[:, sl], hG[:, sl])
                nc.sync.dma_start_transpose(bhGT[:, sl], bhG[:, sl])

            P = pp.tile([H, 512], F32, tag="p")
            nc.tensor.matmul(P[:, :N], lhsT=w_t["WC"][:], rhs=eT_t[:, :N],
                             start=True, stop=False)
            nc.tensor.matmul(P[:, :N], lhsT=w_t["WA"][:], rhs=hGT[:, :N],
                             start=False, stop=False)
            nc.tensor.matmul(P[:, :N], lhsT=ident_bf[:], rhs=bhGT[:, :N],
                             start=False, stop=True)

            ep_sb = wpool.tile([H, 512], BF, tag="ep")
            nc.scalar.activation(ep_sb[:, :N], P[:, :N], AF.Copy,
                                 accum_out=esum_strip[:, gi : gi + 1])
            epre_writes.append(
                nc.scalar.dma_start(epre_d[:, j0 : j0 + N], ep_sb[:, :N]).ins)

            dummy = wpool.tile([H, 512], BF, tag="dummy")
            nc.vector.scalar_tensor_tensor(
                out=dummy[:, :N], in0=ep_sb[:, :N], scalar=0.0,
                in1=ep_sb[:, :N], op0=AL.add, op1=AL.mult,
                accum_out=esq_strip[:, gi : gi + 1])

            gates = wpool.tile([H, 512], BF, tag="gates")
            nc.scalar.activation(gates[:, :N], P[:, :N], AF.Sigmoid, bias=ceb)

            Q = pq.tile([H, 512], F32, tag="q")
            nc.tensor.matmul(Q[:, :N], lhsT=w_t["WV"][:], rhs=hGT[:, :N],
                             start=True, stop=True)
            msgT = wpool.tile([H, 512], BF, tag="msgT")
            nc.vector.scalar_tensor_tensor(
                out=msgT[:, :N], in0=Q[:, :N], scalar=bV, in1=gates[:, :N],
                op0=AL.add, op1=AL.mult)
            msg_e = wpool.tile([CHUNK, 512], BF, tag="msg_e")
            oh = wpool.tile([CHUNK, 512], BF, tag="oh")
            for c in range(k):
                sl = slice(c * CHUNK, (c + 1) * CHUNK)
                nc.sync.dma_start_transpose(msg_e[:, sl], msgT[:, sl])
                nc.vector.tensor_scalar(
                    out=oh[:, sl], in0=iota_bf[:],
                    scalar1=rrel_t[:, ch0 + c : ch0 + c + 1], scalar2=None,
                    op0=AL.is_equal)
                nc.tensor.matmul(hn_ps[:, :H], lhsT=oh[:, sl], rhs=msg_e[:, sl],
                                 start=False, stop=(last and c == k - 1))
            if last:
                hn129 = wpool.tile([CHUNK, 130], BF, tag="hn129")
                nc.vector.memset(hn129[:, 0:1], 1.0)
                nc.scalar.activation(hn129[:, 1 : 1 + H], hn_ps[:, :H], AF.Copy)
                hn_writes[g] = nc.scalar.dma_start(
                    hn_d[g * H : (g + 1) * H, :], hn129[:, 1 : 1 + H]).ins
                nc.tensor.matmul(hstat_ps[:, : H + 1],
                                 lhsT=hn129[:, 1 : 1 + H],
                                 rhs=hn129[:, 0 : 1 + H],
                                 start=(g == 0), stop=(g == NT - 1))

        epre_join = nc.sync.nop(nofuse=True, hint="epre_join").ins
        for w in epre_writes:
            add_dep_helper(epre_join, w, reason="epre before pass 2")

        # phase S
        sv = cpool.tile([H, 24], F32, tag="statv")
        stats4 = cpool.tile([H, 4], F32, tag="stats4")
        nc.vector.tensor_reduce(stats4[:, 0:1], esum_strip[:],
                                axis=mybir.AxisListType.X, op=AL.add)
        nc.vector.tensor_reduce(stats4[:, 1:2], esq_strip[:],
                                axis=mybir.AxisListType.X, op=AL.add)
        hstat_sb = cpool.tile([H, 256], F32, tag="hstat_sb")
        nc.scalar.activation(hstat_sb[:, : H + 1], hstat_ps[:, : H + 1], AF.Copy)
        nc.vector.tensor_copy(stats4[:, 2:3], hstat_sb[:, 0:1])
        junk = wpool.tile([H, 512], BF, tag="dummy")
        nc.vector.scalar_tensor_tensor(
            out=junk[:, :H], in0=hstat_sb[:, 1 : 1 + H], scalar=0.0,
            in1=ident_f[:], op0=AL.add, op1=AL.mult,
            accum_out=stats4[:, 3:4])

        stats8 = cpool.tile([H, 4], F32, tag="stats8")
        cc_sem = nc.alloc_semaphore("cc_sem")
        dma_sem = nc.alloc_semaphore("cc_dma_sem")
        with tc.tile_critical():
            nc.gpsimd.dma_start(out=stats_in_d[:, :], in_=stats4[:]).then_inc(
                dma_sem, 16)
            nc.gpsimd.wait_ge(dma_sem, 16)
            nc.gpsimd.collective_compute(
                "AllReduce", AL.add,
                replica_groups=[list(range(N_CORES))],
                ins=[stats_in_d[:, :]], outs=[stats_out_d[:, :]],
            ).then_inc(cc_sem, 1)
            nc.gpsimd.wait_ge(cc_sem, 1)
            nc.gpsimd.dma_start(out=stats8[:], in_=stats_out_d[:, :]).then_inc(
                dma_sem, 16)
            nc.gpsimd.wait_ge(dma_sem, 32)

        def bn_vec(sum_c, sq_c, inv_n, gam, bet, sc_out, sh_out):
            mean = sv[:, 8 + sc_out : 9 + sc_out]
            var = sv[:, 10 + sc_out : 11 + sc_out]
            m2 = sv[:, 12 + sc_out : 13 + sc_out]
            tmp = sv[:, 14 + sc_out : 15 + sc_out]
            nc.vector.tensor_scalar(mean, stats8[:, sum_c : sum_c + 1], inv_n,
                                    None, op0=AL.mult)
            nc.vector.tensor_scalar(var, stats8[:, sq_c : sq_c + 1], inv_n,
                                    None, op0=AL.mult)
            nc.scalar.activation(m2, mean, AF.Square)
            nc.vector.tensor_sub(var, var, m2)
            rstd = sv[:, 16 + sc_out : 17 + sc_out]
            std = sv[:, 18 + sc_out : 19 + sc_out]
            nc.scalar.activation(std, var, AF.Sqrt, bias=eps_v)
            nc.vector.reciprocal(rstd, std)
            nc.vector.tensor_mul(sv[:, sc_out : sc_out + 1], gam, rstd)
            nc.vector.tensor_mul(tmp, mean, sv[:, sc_out : sc_out + 1])
            nc.vector.tensor_sub(sv[:, sh_out : sh_out + 1], bet, tmp)

        bn_vec(0, 1, 1.0 / E, g_e, b_e, 0, 1)
        bn_vec(2, 3, 1.0 / V, g_h, b_h, 2, 3)
        sc_e, sh_e = sv[:, 0:1], sv[:, 1:2]

        screp_ps = pq.tile([H, 512], F32, tag="q")
        nc.tensor.transpose(screp_ps[:, :H], sv[:, 2:3].to_broadcast([H, H]),
                            ident_f[:])
        screp = cpool.tile([H, H], F32, tag="screp")
        nc.scalar.activation(screp[:], screp_ps[:, :H], AF.Copy)
        shrep_ps = pq.tile([H, 512], F32, tag="q")
        nc.tensor.transpose(shrep_ps[:, :H], sv[:, 3:4].to_broadcast([H, H]),
                            ident_f[:])
        shrep = cpool.tile([H, H], F32, tag="shrep")
        nc.scalar.activation(shrep[:], shrep_ps[:, :H], AF.Copy)

        # phase H
        for g in range(NT):
            hn_sb = opool.tile([CHUNK, H], BF, tag="hn_sb")
            ins = nc.sync.dma_start(hn_sb[:], hn_d[g * H : (g + 1) * H, :])
            add_dep_helper(ins.ins, hn_writes[g], reason="hn RAW")
            hl_sb = opool.tile([CHUNK, H], F32, tag="hl_sb")
            nc.sync.dma_start(hl_sb[:], h_loc_d[g * H : (g + 1) * H, :])
            t1 = opool.tile([CHUNK, H], F32, tag="t1")
            nc.vector.tensor_mul(t1[:], hn_sb[:], screp[:])
            nc.vector.tensor_add(t1[:], t1[:], shrep[:])
            nc.vector.tensor_scalar_max(t1[:], t1[:], 0.0)
            xo = opool.tile([CHUNK, H], F32, tag="xo")
            nc.vector.tensor_add(xo[:], t1[:], hl_sb[:])
            nc.scalar.dma_start(xout_d[g * H : (g + 1) * H, :], xo[:])

        # phase E2
        ch = 0
        while ch < n_chunks:
            k = min(GROUP, n_chunks - ch)
            N = k * CHUNK
            j0 = ch * CHUNK
            ep2 = opool.tile([H, 512], BF, tag="ep2")
            ins = nc.sync.dma_start(ep2[:, :N], epre_d[:, j0 : j0 + N])
            add_dep_helper(ins.ins, epre_join, reason="epre RAW")
            enr = opool.tile([H, 512], BF, tag="enr")
            nc.scalar.activation(enr[:, :N], ep2[:, :N], AF.Relu,
                                 bias=sh_e, scale=sc_e)
            W2 = pp.tile([H, 512], F32, tag="p")
            nc.tensor.matmul(W2[:, :N], lhsT=w_t["Wo"][:], rhs=enr[:, :N],
                             start=True, stop=True)
            eT2 = opool.tile([H, 512], BF, tag="eT2")
            nc.sync.dma_start(eT2[:, :N], eT_d[:, j0 : j0 + N])
            eo = opool.tile([H, 512], F32, tag="eo")
            nc.vector.scalar_tensor_tensor(
                out=eo[:, :N], in0=W2[:, :N], scalar=c2, in1=eT2[:, :N],
                op0=AL.add, op1=AL.add)
            nc.scalar.dma_start(eoutT_d[:, j0 : j0 + N], eo[:, :N])
            ch += k

    nc.compile()
    return nc


_CACHE = {}
_LAST = None


def kernel(h, e, time_emb, edge_index, WU, bU, WV, bV, WA, bA, WC, bC,
           Wt, bt, Wo, bo, WB, bB, g_h, b_h, g_e, b_e):
    h = np.asarray(h, dtype=F)
    e = np.asarray(e, dtype=F)
    V, E = h.shape[0], e.shape[0]

    plan = _build_plan(edge_index, V)
    meta = (V, E, plan.NT, plan.VT_PAD, plan.n_chunks, plan.E_pad,
            tuple(map(int, plan.tile_chunks)))
    if meta not in _CACHE:
        _CACHE[meta] = _build_bass(dict(
            V=V, E=E, NT=plan.NT, VT_PAD=plan.VT_PAD,
            n_chunks=plan.n_chunks, E_pad=plan.E_pad,
            tile_chunks=list(map(int, plan.tile_chunks))))
    nc = _CACHE[meta]

    h_ext = np.zeros((V + 8, H), dtype=np.float32)
    h_ext[:V] = h
    h_ext = _bf16(h_ext)
    w_bf = {k: _bf16(v) for k, v in
            dict(WU=WU, WV=WV, WA=WA, WB=WB, WC=WC, Wo=Wo).items()}

    ceb = F(bA) + F(bB) + F(bC)
    temb = F(time_emb) @ F(Wt) + F(bt)
    c2 = temb @ F(Wo) + F(bo)
    vecs = np.zeros((H, 8), F)
    for i, v in enumerate([ceb, F(bV), c2, F(g_e), F(b_e), F(g_h), F(b_h),
                           np.full(H, EPS, F)]):
        vecs[:, i] = v

    VS, VT_PAD, E_pad = plan.VS, plan.VT_PAD, plan.E_pad
    in_maps = []
    for c in range(N_CORES):
        cp = plan.cores[c]
        real = cp.orig >= 0
        e_pad = np.zeros((E_pad, H), dtype=np.float32)
        e_pad[real] = e[cp.orig[real]]
        eT = np.ascontiguousarray(_bf16(e_pad).T)
        h_loc = np.zeros((VT_PAD, H), dtype=np.float32)
        h_loc[:VS] = h[c * VS : (c + 1) * VS]
        m = dict(eT=eT, h_ext=h_ext,
                 hT_loc=np.ascontiguousarray(_bf16(h_loc).T),
                 h_loc=np.ascontiguousarray(h_loc),
                 col_idx=cp.col_idx, lr_idx=cp.lr_idx, r_rel=cp.r_rel,
                 vecs=vecs, **w_bf)
        in_maps.append(m)

    global _LAST
    _LAST = (nc, in_maps)
    res = run_bass_kernel_spmd(nc, in_maps, list(range(N_CORES)))

    x_out = np.zeros((V, H), F)
    e_out = np.zeros((E, H), F)
    for c in range(N_CORES):
        cp = plan.cores[c]
        r = res.results[c]
        x_out[c * VS : (c + 1) * VS] = r["x_out"][:VS]
        real = cp.orig >= 0
        e_out[cp.orig[real]] = r["eoutT"].T[real]
    return x_out, e_out
